# revision 20
# baseline (speedup 1.0000x reference)
"""Trainium2 Bass kernel for nn_Encoder (ragged embedding-bag + masked LSTM).

Strategy (data-parallel over batch, 8 cores, B=256 -> 32 batches/core):

Host preprocessing (cheap, index-only):
  * The ragged embedding bag obs_sum = segment_sum(obs_emb[obs_ids], obs_slot)
    collapses to a matmul: obs_sum = counts @ obs_emb where counts[s, v] is a
    histogram over the (sorted) slots -- computed with one np.bincount.
  * The action gather is a one-hot matmul against action_emb.  Both fuse into
    one coefficient matrix C [B*T, 16] and a 16x32 table so that
    embedded = C @ table (computed on-device in fp32).
  * The LSTM input projection folds through the embedding:
    x_t @ W_ih.T = C_t @ (table @ W_ih.T), so the device never materializes
    embedded for the LSTM; an augmented C (bias column + "dead" column) also
    injects per-(b,t) length masking: for t >= len_b the dead column adds
    -1e4 / +1e4 / 0 / -1e4 to the i/f/g/o gate pre-activations, forcing
    i=0, f=1, o=0 -> cell frozen, h emitted as exact 0 (matches packed
    semantics), with no per-step mask tensors.

Device program (same SPMD program on all 8 cores, per-core data differs):
  * Phase 1: embedded output pass: emb = C @ table in fp32 via TensorE,
    PSUM -> DRAM directly.
  * Phase 2: LSTM over T_run = max(len) steps, batch 32 per core.
    Gate layout: PSUM [128 (gate dims), 4 gates, 16 steps, 32 batch].
    - ih gate contributions for a 16-step window are precomputed in a burst
      of matmuls (W~ stationary [18,128] per gate packed in PE row-groups,
      C^T window moving) directly into PSUM with start=True.
    - Per step, h @ W_hh.T accumulates into the same PSUM slice (start=False)
      using bf16x2 compensated weights (W = hi + lo) for ~5e-4 abs accuracy.
    - One Sigmoid ACT over all four gates (g pre-scaled by 2 in the weights so
      tanh(g) = 2*sigmoid(2g) - 1), then a fused DVE chain updates ctil = 2*c:
        v = (s_g - 0.5) * s_i ; u = s_f * ctil ; ctil = 4*v + u
      one Tanh ACT gives tanh(c) = tanh(0.5*ctil), and h = s_o * tanh(c) is
      written twice (bf16 for the next matmul, fp32 into the output history).
    - h history flushes to DRAM every 64 steps.
  * Final h is gathered on host from outputs[b, len_b-1]; final c = ctil/2.
"""

import numpy as np
import ml_dtypes

import concourse.bass as bass
import concourse.bacc as bacc
import concourse.mybir as mybir
import concourse.tile as tile
from concourse.bass_utils import run_bass_kernel_spmd

BF16 = ml_dtypes.bfloat16

B, T, E, H = 256, 1024, 32, 128
N_OBS_VOCAB = 11
N_ACT_VOCAB = 5
NCORES = 8
BC = B // NCORES          # batches per core
BTC = BC * T              # slots per core
KTAB = 16                 # obs vocab (11) + act vocab (5)
KAUG = 18                 # + bias row + death row
G4 = 4 * H                # 512
SW = 16                   # steps per ih-precompute window (one PSUM bank/gate)
FW = 64                   # steps per output flush
EMBG = 2048               # slots per emb-pass group (16 chunks of 128)
DEAD = 1.0e4

F32 = mybir.dt.float32
BF16_DT = mybir.dt.bfloat16
AF = mybir.ActivationFunctionType
ALU = mybir.AluOpType

_PROGRAM_CACHE: dict = {}


def _build_program(T_run: int):
    nc = bacc.Bacc("TRN2", target_bir_lowering=False, debug=False)

    n_win = (T_run + SW - 1) // SW

    # ---- DRAM I/O (per-core shapes) ----
    ct_bf = nc.dram_tensor("ct_bf", [KAUG, T_run * BC], BF16_DT, kind="ExternalInput")
    ct_f32 = nc.dram_tensor("ct_f32", [KTAB, BTC], F32, kind="ExternalInput")
    table_d = nc.dram_tensor("table_d", [KTAB, E], F32, kind="ExternalInput")
    whh_d = nc.dram_tensor("whh_d", [H, 2 * G4], BF16_DT, kind="ExternalInput")
    wtil_d = nc.dram_tensor("wtil_d", [KAUG, 2 * G4], BF16_DT, kind="ExternalInput")

    # h-major layout: a [H, t, b] SBUF tile cannot DMA into b-major [b, t, H]
    # DRAM (would need >3 AP dims / non-contiguous final dim); host transposes.
    outs_d = nc.dram_tensor("outs_d", [H, T, BC], F32, kind="ExternalOutput")
    emb_d = nc.dram_tensor("emb_d", [BTC, E], F32, kind="ExternalOutput")
    cfin_d = nc.dram_tensor("cfin_d", [H, BC], F32, kind="ExternalOutput")

    with tile.TileContext(nc) as tc:
        # ================= Phase 1: embedded output =================
        with (
            tc.tile_pool(name="embc", bufs=1) as embc,
            tc.tile_pool(name="embs", bufs=2) as embs,
            tc.tile_pool(name="embp", bufs=2, space="PSUM") as embp,
        ):
            table_sb = embc.tile([KTAB, E], F32)
            nc.sync.dma_start(table_sb[:], table_d.ap())
            n_grp = BTC // EMBG
            for grp in range(n_grp):
                stage = embs.tile([KTAB, EMBG], F32, tag="ctstage")
                nc.sync.dma_start(
                    stage[:], ct_f32.ap()[:, grp * EMBG : (grp + 1) * EMBG]
                )
                pse = embp.tile([128, EMBG // 128, E], F32, tag="pse")
                for cc in range(EMBG // 128):
                    nc.tensor.matmul(
                        pse[:, cc, :],
                        stage[:, cc * 128 : (cc + 1) * 128],
                        table_sb[:],
                        start=True,
                        stop=True,
                    )
                esb = embs.tile([128, EMBG // 128, E], F32, tag="esb")
                nc.vector.tensor_copy(esb[:], pse[:])
                dst = emb_d.ap()[grp * EMBG : (grp + 1) * EMBG, :].rearrange(
                    "(cc p) e -> p cc e", p=128
                )
                nc.sync.dma_start(dst, esb[:])

        # ================= Phase 2: LSTM =================
        with (
            tc.tile_pool(name="wpool", bufs=1) as wpool,
            tc.tile_pool(name="state", bufs=1) as state,
            tc.tile_pool(name="ctp", bufs=2) as ctp,
            tc.tile_pool(name="ewp", bufs=3) as ewp,
            tc.tile_pool(name="hist", bufs=2) as hist,
            tc.tile_pool(name="psw", bufs=2, space="PSUM") as psw,
        ):
            whh_sb = wpool.tile([H, 2 * G4], BF16_DT)
            nc.sync.dma_start(whh_sb[:], whh_d.ap())
            wtil_sb = wpool.tile([KAUG, 2 * G4], BF16_DT)
            nc.sync.dma_start(wtil_sb[:], wtil_d.ap())

            ctil = state.tile([H, BC], F32)   # 2*c, running cell state
            hbf = state.tile([H, BC], BF16_DT)  # h in bf16 for the matmul
            nc.vector.memset(ctil[:], 0.0)
            nc.vector.memset(hbf[:], 0.0)

            ctr_cur = None
            hh = None

            def dma_ct_window(w):
                sww = min(SW, T_run - w * SW)
                t = ctp.tile([KAUG, SW * BC], BF16_DT, tag="ctr", name=f"ctr{w}")
                nc.sync.dma_start(
                    t[:, : sww * BC],
                    ct_bf.ap()[:, w * SW * BC : w * SW * BC + sww * BC],
                )
                return t

            ctr_cur = dma_ct_window(0)
            for w in range(n_win):
                sww = min(SW, T_run - w * SW)
                ctr_next = dma_ct_window(w + 1) if w + 1 < n_win else None

                ps = psw.tile([128, 4, SW, BC], F32, tag="ps", name=f"ps{w}")
                for g in range(4):
                    for half in range(2):
                        nc.tensor.matmul(
                            ps[:, g, :sww, :],
                            wtil_sb[:, half * G4 + g * H : half * G4 + (g + 1) * H],
                            ctr_cur[:, : sww * BC],
                            start=(half == 0),
                            stop=False,
                            skip_group_check=True,
                        )

                for j in range(sww):
                    t = w * SW + j
                    for g in range(4):
                        nc.tensor.matmul(
                            ps[:, g, j, :],
                            whh_sb[:, g * 128 : (g + 1) * 128],
                            hbf[:],
                            start=False,
                            stop=False,
                            skip_group_check=True,
                        )
                        nc.tensor.matmul(
                            ps[:, g, j, :],
                            whh_sb[:, G4 + g * 128 : G4 + (g + 1) * 128],
                            hbf[:],
                            start=False,
                            stop=True,
                            skip_group_check=True,
                        )
                    s4 = ewp.tile([128, 4, BC], F32, tag="s4", name=f"s4_{t}")
                    nc.scalar.activation(s4[:], ps[:, :, j, :], AF.Sigmoid)
                    v_t = ewp.tile([H, BC], F32, tag="v", name=f"v{t}")
                    nc.vector.scalar_tensor_tensor(
                        v_t[:], s4[:, 2, :], 0.5, s4[:, 0, :], ALU.subtract, ALU.mult
                    )
                    u_t = ewp.tile([H, BC], F32, tag="u", name=f"u{t}")
                    nc.vector.tensor_mul(u_t[:], s4[:, 1, :], ctil[:])
                    nc.vector.scalar_tensor_tensor(
                        ctil[:], v_t[:], 4.0, u_t[:], ALU.mult, ALU.add
                    )
                    tc_t = ewp.tile([H, BC], F32, tag="tc", name=f"tc{t}")
                    nc.scalar.activation(tc_t[:], ctil[:], AF.Tanh, scale=0.5)
                    if t % FW == 0:
                        hh = hist.tile([H, FW * BC], F32, tag="hh", name=f"hh{t}")
                    nc.vector.tensor_mul(hbf[:], s4[:, 3, :], tc_t[:])
                    nc.vector.tensor_mul(
                        hh[:, (t % FW) * BC : (t % FW + 1) * BC], s4[:, 3, :], tc_t[:]
                    )
                    if t % FW == FW - 1 or t == T_run - 1:
                        nfl = t % FW + 1
                        t0 = t - nfl + 1
                        nc.sync.dma_start(
                            outs_d.ap()[:, t0 : t0 + nfl, :], hh[:, : nfl * BC]
                        )
                ctr_cur = ctr_next

            nc.sync.dma_start(cfin_d.ap(), ctil[:])

    nc.compile()
    return nc


def _prepare_host(inputs):
    obs_ids = np.ascontiguousarray(inputs["obs_ids"]).astype(np.int64)
    obs_slot = np.ascontiguousarray(inputs["obs_slot"]).astype(np.int64)
    action_ids = np.ascontiguousarray(inputs["action_ids"]).astype(np.int64)
    is_action = np.ascontiguousarray(inputs["is_action"]).astype(bool)
    lengths = np.ascontiguousarray(inputs["input_lengths"]).astype(np.int64)
    action_emb = np.asarray(inputs["action_emb"], np.float32)
    obs_emb = np.asarray(inputs["obs_emb"], np.float32)
    W_ih = np.asarray(inputs["W_ih"], np.float32)
    W_hh = np.asarray(inputs["W_hh"], np.float32)
    bias = (np.asarray(inputs["b_ih"], np.float32)
            + np.asarray(inputs["b_hh"], np.float32))

    T_run = int(lengths.max())

    # counts/one-hot coefficient matrix C [B*T, 16]
    cnt = np.bincount(
        obs_slot * N_OBS_VOCAB + obs_ids, minlength=B * T * N_OBS_VOCAB
    ).reshape(B * T, N_OBS_VOCAB)
    ia = is_action.reshape(-1)
    aid = action_ids.reshape(-1)
    Cm = np.zeros((B * T, KTAB), np.float32)
    Cm[:, :N_OBS_VOCAB] = np.where(ia[:, None], 0, cnt)
    Cm[np.arange(B * T), N_OBS_VOCAB + aid] = np.where(ia, 1.0, Cm[np.arange(B * T), N_OBS_VOCAB + aid])

    table = np.concatenate([obs_emb, action_emb], 0)          # [16, 32]

    # gate scaling: g rows x2 (sigmoid-only trick)
    gscale = np.ones((G4,), np.float32)
    gscale[2 * H : 3 * H] = 2.0

    # W~ augmented [18, 512]: table@W_ih.T, bias row, death row
    Wtil = np.zeros((KAUG, G4), np.float32)
    Wtil[:KTAB] = table @ W_ih.T
    Wtil[KTAB] = bias
    Wtil[KTAB + 1, 0 * H : 1 * H] = -DEAD      # i -> 0
    Wtil[KTAB + 1, 1 * H : 2 * H] = +DEAD      # f -> 1
    Wtil[KTAB + 1, 3 * H : 4 * H] = -DEAD      # o -> 0
    Wtil *= gscale[None, :]

    Wtil_hi = Wtil.astype(BF16).astype(np.float32)
    Wtil_lo = (Wtil - Wtil_hi).astype(BF16)
    # stationary image [18, 1024]: cols 0:512 hi (gate-major), 512:1024 lo
    wtil_img = np.concatenate([Wtil_hi.astype(BF16), Wtil_lo], axis=1)

    # W_hh stationaries [128, 1024]: cols g*128 hi (lhsT = W_hh_g.T * gscale), then lo
    Whh_s = W_hh * gscale[:, None]
    Whh_sT = Whh_s.T                                    # [H, 512] lhsT layout
    Whh_hi = Whh_sT.astype(BF16).astype(np.float32)
    Whh_lo = (Whh_sT - Whh_hi).astype(BF16)
    whh_img = np.concatenate([Whh_hi.astype(BF16), Whh_lo], axis=1)  # [128, 1024]

    # per-core C arrays
    dead = (np.arange(T)[None, :] >= lengths[:, None])  # [B, T]
    in_maps = []
    for k in range(NCORES):
        sl = slice(k * BC * T, (k + 1) * BC * T)
        Ck = Cm[sl].reshape(BC, T, KTAB)
        # t-major augmented bf16 [18, T_run*BC]
        caug = np.zeros((KAUG, T_run, BC), np.float32)
        caug[:KTAB] = Ck[:, :T_run, :].transpose(2, 1, 0)
        caug[KTAB] = 1.0
        caug[KTAB + 1] = dead[k * BC : (k + 1) * BC, :T_run].T.astype(np.float32)
        ct_bf = np.ascontiguousarray(caug.reshape(KAUG, T_run * BC).astype(BF16))
        # b-major fp32 [16, BC*T]
        ct_f32 = np.ascontiguousarray(Ck.reshape(BTC, KTAB).T)
        in_maps.append(
            dict(
                ct_bf=ct_bf,
                ct_f32=ct_f32,
                table_d=np.ascontiguousarray(table),
                whh_d=np.ascontiguousarray(whh_img),
                wtil_d=np.ascontiguousarray(wtil_img),
            )
        )
    return in_maps, T_run, lengths


def _postprocess(results, lengths):
    outputs = np.concatenate(
        [np.ascontiguousarray(r["outs_d"].transpose(2, 1, 0)) for r in results], axis=0
    )  # [B, T, H]
    embedded = np.concatenate([r["emb_d"].reshape(BC, T, E) for r in results], axis=0)
    c = np.concatenate([0.5 * r["cfin_d"].T for r in results], axis=0)[None]  # [1,B,H]
    h = outputs[np.arange(B), lengths - 1][None]                      # [1,B,H]
    return outputs, h.copy(), c, embedded


def _run(inputs, **spmd_kwargs):
    in_maps_data, T_run, lengths = _prepare_host(inputs)
    if T_run not in _PROGRAM_CACHE:
        _PROGRAM_CACHE[T_run] = _build_program(T_run)
    nc = _PROGRAM_CACHE[T_run]
    res = run_bass_kernel_spmd(nc, in_maps_data, list(range(NCORES)), **spmd_kwargs)
    return _postprocess(res.results, lengths), res


def kernel(**inputs):
    out, _ = _run(inputs)
    return out


if __name__ == "__main__":
    # small-scale CoreSim selftest against a numpy emulation
    import sys
    from concourse.bass_interp import CoreSim

    T_small = int(sys.argv[1]) if len(sys.argv) > 1 else 40

    rng = np.random.default_rng(0)
    fake = dict(
        obs_ids=rng.integers(0, N_OBS_VOCAB, B * T).astype(np.int32),
        obs_slot=np.sort(rng.integers(0, B * T, B * T)).astype(np.int32),
        action_ids=rng.integers(0, N_ACT_VOCAB, (B, T)).astype(np.int32),
        is_action=rng.random((B, T)) < 0.5,
        input_lengths=np.sort(rng.integers(max(2, T_small // 4), T_small + 1, B))[::-1].astype(np.int32),
        action_emb=rng.normal(size=(N_ACT_VOCAB, E)).astype(np.float32),
        obs_emb=rng.normal(size=(N_OBS_VOCAB, E)).astype(np.float32),
        W_ih=rng.uniform(-0.09, 0.09, (G4, E)).astype(np.float32),
        W_hh=rng.uniform(-0.09, 0.09, (G4, H)).astype(np.float32),
        b_ih=rng.uniform(-0.09, 0.09, G4).astype(np.float32),
        b_hh=np.zeros(G4, np.float32),
    )

    in_maps_data, T_run, lengths = _prepare_host(fake)
    print(f"T_run={T_run}; building program...")
    import time as _time

    t0 = _time.time()
    nc = _build_program(T_run)
    n_inst = sum(len(b.instructions) for b in nc.m.functions[0].blocks)
    print(f"build+schedule time: {_time.time()-t0:.1f}s; instructions: {n_inst}")

    # numpy expected for core 0
    def expect(core):
        Wih, Whh = fake["W_ih"], fake["W_hh"]
        bias = fake["b_ih"] + fake["b_hh"]
        table = np.concatenate([fake["obs_emb"], fake["action_emb"]], 0)
        cnt = np.bincount(
            fake["obs_slot"].astype(np.int64) * N_OBS_VOCAB + fake["obs_ids"],
            minlength=B * T * N_OBS_VOCAB,
        ).reshape(B * T, N_OBS_VOCAB)
        ia = fake["is_action"].reshape(-1)
        Cm = np.zeros((B * T, KTAB), np.float32)
        Cm[:, :N_OBS_VOCAB] = np.where(ia[:, None], 0, cnt)
        idx = np.arange(B * T)
        Cm[idx, N_OBS_VOCAB + fake["action_ids"].reshape(-1)] = np.where(
            ia, 1.0, Cm[idx, N_OBS_VOCAB + fake["action_ids"].reshape(-1)]
        )
        emb = (Cm @ table).reshape(B, T, E)[core * BC : (core + 1) * BC]
        L = lengths[core * BC : (core + 1) * BC]
        sig = lambda x: 1 / (1 + np.exp(-x))
        h = np.zeros((BC, H), np.float32)
        c = np.zeros((BC, H), np.float32)
        outs = np.zeros((BC, T, H), np.float32)
        for t in range(T_run):
            g = h @ Whh.T + emb[:, t] @ Wih.T + bias
            i, f, gg, o = np.split(g, 4, -1)
            i, f, o = sig(i), sig(f), sig(o)
            gg = np.tanh(gg)
            m = (t < L)[:, None].astype(np.float32)
            cn = f * c + i * gg
            hn = o * np.tanh(cn)
            h = m * hn + (1 - m) * h
            c = m * cn + (1 - m) * c
            outs[:, t] = hn * m
        return outs, h, c, emb

    core = 0
    sim = CoreSim(nc)
    for name, arr in in_maps_data[core].items():
        sim.tensor(name)[:] = arr
    # hardware gets zero-donated output buffers; emulate that
    for name in ("outs_d", "emb_d", "cfin_d"):
        sim.tensor(name)[:] = 0.0
    t0 = _time.time()
    sim.simulate()
    print(f"sim time: {_time.time()-t0:.1f}s  sim ns: {sim.time}")

    outs_e, h_e, c_e, emb_e = expect(core)
    outs_a = sim.tensor("outs_d").transpose(2, 1, 0)
    emb_a = sim.tensor("emb_d").reshape(BC, T, E)
    c_a = 0.5 * sim.tensor("cfin_d").T
    print("outs err:", np.abs(outs_a - outs_e).max())
    print("emb  err:", np.abs(emb_a - emb_e).max())
    print("c    err:", np.abs(c_a - c_e).max())
